# revision 33
# baseline (speedup 1.0000x reference)
"""Multi-head attention (B=2, N=2048, D=1024, H=16) on 8 Trainium2 cores.

Sharding: data-parallel over batch (2) x tensor-parallel over head groups (4).
Core c handles batch c//4, heads 4*(c%4) .. 4*(c%4)+3.

Per-core kernel (ACT exp is the long pole; PE kept just under it):
  - x and all projection weights stream in as bf16 (x split per k-tile so
    the first chains start ~3us in); projections matmul in bf16 at full
    rate, accumulate f32, drain to f32r q/k with the bias added on DVE.
  - v stored as [v|1] bf16 (ones column via the bias row), so PV
    accumulates softmax sums in column 64 for free.
  - S^T = kT^T qT in f32r, exp on ACT -> P^T bf16.
  - PV "flipped": P^T tiles are the stationary operand, [v|1] (65 cols)
    the moving one -> 65 cycles per (key-tile, query-ptile) instead of
    512; O lands [query, 65] with queries on partitions.
  - normalize on DVE with per-partition 1/sums, then one XBAR DMA
    transpose per (head, query-tile) puts O^T into SBUF for the out-proj.
  - out = O^T^T Wo in bf16, K=64 chains per 128-channel block.
Host: out[b] = sum of the 4 group partials + b_o.
"""

import sys

sys.path.insert(0, "/opt/trn_rl_repo")

import numpy as np
import ml_dtypes

B, N, D, H = 2, 2048, 1024, 16
SUB = D // H  # 64
GROUPS = 4  # tensor-parallel head groups
NH = H // GROUPS  # 4 local heads per core
CH = NH * SUB  # 256 local channels
NCORES = 8
VW = SUB + 1  # 65: per-head [v|1] width


def build_nc(NT=N, DK=D, DO=D, nh=NH, name="mha"):
    import concourse.mybir as mybir
    from concourse import bacc
    from concourse.tile import TileContext

    f32 = mybir.dt.float32
    f32r = mybir.dt.float32r
    bf16 = mybir.dt.bfloat16
    Exp = mybir.ActivationFunctionType.Exp

    sub = 64
    ch = nh * sub  # 256
    KT = DK // 128  # 8 contraction ptiles
    CHT = ch // 128  # 2 channel ptiles
    TOKT = NT // 128  # 16 token/key ptiles
    QT = NT // 512  # 4 query tiles
    scale = sub ** -0.5

    nc = bacc.Bacc(None, name=name)
    xTd = nc.dram_tensor("xT", [128, NT // 256, KT, 256], bf16, kind="ExternalInput")
    wqd = nc.dram_tensor("wq", [128, KT, ch], bf16, kind="ExternalInput")
    wkd = nc.dram_tensor("wk", [128, KT, ch], bf16, kind="ExternalInput")
    wvd = nc.dram_tensor("wv", [128, KT, nh * VW], bf16, kind="ExternalInput")
    auxd = nc.dram_tensor("aux", [1, 128 + nh * VW], bf16, kind="ExternalInput")
    wod = nc.dram_tensor("wo", [ch, DO], bf16, kind="ExternalInput")
    bqkd = nc.dram_tensor("bqk", [128, 2, CHT], f32, kind="ExternalInput")
    out = nc.dram_tensor("out", [NT, DO], bf16, kind="ExternalOutput")

    with TileContext(nc) as tc:
        with tc.tile_pool(name="persist", bufs=1) as pp:
            xT = pp.tile([128, NT // 256, KT, 256], bf16)
            wq_sb = pp.tile([128, KT, ch], bf16)
            wk_sb = pp.tile([128, KT, ch], bf16)
            wv_sb = pp.tile([128, KT, nh * VW], bf16)
            aux_sb = pp.tile([1, 128 + nh * VW], bf16)
            qT_sb = pp.tile([128, CHT, NT], f32r)
            kT_sb = pp.tile([128, CHT, NT], f32r)
            v1 = pp.tile([128, TOKT, nh * VW], bf16)
            # O^T staging: token t of qt decomposes as (qt, qi, p)
            oT_sb = pp.tile([128, CHT, QT, 4, 128], bf16)
            wo_sb = pp.tile([128, CHT, DO], bf16)
            bqk_sb = pp.tile([128, 2, CHT], f32)
            zeros16 = pp.tile([128, 128], bf16)
            dumm16 = pp.tile([128, nh * VW], bf16)

            nc.sync.dma_start(bqk_sb[:], bqkd[:])
            nc.sync.dma_start(wk_sb[:], wkd[:])
            for c in range(2):
                nc.sync.dma_start(xT[:, c, :, :], xTd[:, c, :, :])
            nc.sync.dma_start(wv_sb[:], wvd[:])
            nc.sync.dma_start(aux_sb[:], auxd[:])
            nc.sync.dma_start(wq_sb[:], wqd[:])
            for c in range(2, NT // 256):
                nc.sync.dma_start(xT[:, c, :, :], xTd[:, c, :, :])
            for ct in range(CHT):
                nc.sync.dma_start(wo_sb[:, ct, :], wod[ct * 128 : (ct + 1) * 128, :])
            zf = pp.tile([128, 128], f32)
            nc.vector.memset(zf[:], 0.0)
            nc.vector.tensor_copy(zeros16[:], zf[:])
            nc.vector.memset(dumm16[:], 0.0)

            # spin the PE during the x DMA so the p-state ramp is done before
            # the first real chains (cold PE runs 3.7x slower)
            with tc.tile_pool(name="wrm", bufs=1, space="PSUM") as wrm:
                wt = wrm.tile([128, 260], f32, name="wt", tag="wt")
                for _ in range(12):
                    nc.tensor.matmul(
                        wt[:], lhsT=zeros16[:], rhs=dumm16[:], start=True, stop=True,
                        skip_group_check=True,
                    )

            with tc.tile_pool(name="stp", bufs=2, space="PSUM") as stp, \
                 tc.tile_pool(name="accp", bufs=2, space="PSUM") as accp, \
                 tc.tile_pool(name="prj", bufs=2, space="PSUM") as prj, \
                 tc.tile_pool(name="ptp", bufs=8) as ptp, \
                 tc.tile_pool(name="nrm", bufs=3) as nrm, \
                 tc.tile_pool(name="osg", bufs=4) as osg:

                def qk_part(nm, mt, ts, ps, k0, k1):
                    w = wq_sb if nm == "q" else wk_sb
                    for kt in range(k0, k1):
                        nc.tensor.matmul(
                            ps[:],
                            lhsT=w[:, kt, mt * 128 : (mt + 1) * 128],
                            rhs=xT[:, 2 * ts : 2 * ts + 2, kt, :],
                            start=(kt == 0),
                            stop=(kt == KT - 1),
                        )
                    if k1 == KT:
                        dst = qT_sb if nm == "q" else kT_sb
                        nc.vector.tensor_scalar_add(
                            dst[:, mt, ts * 512 : (ts + 1) * 512],
                            ps[:],
                            bqk_sb[:, 0 if nm == "q" else 1, mt : mt + 1],
                        )

                def qk_chain(nm, mt, ts):
                    ps = prj.tile([128, 512], f32, name="ps", tag="prj")
                    qk_part(nm, mt, ts, ps, 0, KT)

                def qk_first(nm, c):
                    # (mt0, ts0) by half-token chunk: starts as soon as its
                    # x chunk lands
                    w = wq_sb if nm == "q" else wk_sb
                    dst = qT_sb if nm == "q" else kT_sb
                    ps = prj.tile([128, 256], f32, name="psf", tag="prj")
                    for kt in range(KT):
                        nc.tensor.matmul(
                            ps[:],
                            lhsT=w[:, kt, 0:128],
                            rhs=xT[:, c, kt, :],
                            start=(kt == 0),
                            stop=(kt == KT - 1),
                        )
                    nc.vector.tensor_scalar_add(
                        dst[:, 0, c * 256 : (c + 1) * 256],
                        ps[:],
                        bqk_sb[:, 0 if nm == "q" else 1, 0:1],
                    )

                def v_chain(tt):
                    ps = prj.tile([128, nh * VW], f32, name="psv", tag="prj")
                    for kt in range(KT):
                        nc.tensor.matmul(
                            ps[:],
                            lhsT=xT[
                                :, tt // 2, kt, (tt % 2) * 128 : (tt % 2) * 128 + 128
                            ],
                            rhs=wv_sb[:, kt, :],
                            start=(kt == 0),
                            stop=False,
                        )
                    nc.tensor.matmul(
                        ps[:],
                        lhsT=aux_sb[:, 0:128],
                        rhs=aux_sb[:, 128 : 128 + nh * VW],
                        start=False,
                        stop=True,
                    )
                    nc.vector.tensor_copy(v1[:, tt, :], ps[:])

                ostg = {}

                def o_mm(tt, nt, ps, ct):
                    nc.tensor.matmul(
                        ps[:],
                        lhsT=oT_sb[:, ct, tt // 4, tt % 4, :],
                        rhs=wo_sb[:, ct, nt * 512 : (nt + 1) * 512],
                        start=(ct == 0),
                        stop=(ct == CHT - 1),
                    )

                def o_begin(tt, nt):
                    ps = prj.tile([128, 512], f32, name="pso", tag="prj")
                    o_mm(tt, nt, ps, 0)
                    return ps

                def o_fin(tt, nt, tail, ps):
                    o_mm(tt, nt, ps, 1)
                    if nt == 0:
                        ostg[tt] = osg.tile([128, DO], bf16, name="stg", tag="stg")
                    stg = ostg[tt]
                    if tail and (tt + nt) % 2 == 0:
                        nc.scalar.copy(stg[:, nt * 512 : (nt + 1) * 512], ps[:])
                    else:
                        nc.vector.tensor_copy(stg[:, nt * 512 : (nt + 1) * 512], ps[:])
                    if nt == DO // 512 - 1:
                        nc.sync.dma_start(out[tt * 128 : (tt + 1) * 128, :], stg[:])
                        del ostg[tt]

                done = set()
                half = {}
                from collections import deque

                pending = deque()

                def need(kind, *a):
                    key = (kind,) + a
                    if key in done:
                        return
                    done.add(key)
                    if kind == "q" or kind == "k":
                        if key in half:
                            ps, k0 = half.pop(key)
                            qk_part(*key, ps, k0, KT)
                        else:
                            qk_chain(*key)
                    elif kind == "v":
                        v_chain(*a)

                oprog = {}

                def emit(item):
                    kind = item[0]
                    if kind == "v":
                        need(*item)
                    elif kind in ("q", "k"):
                        key = item
                        if key in done:
                            return
                        if key in half:
                            ps, k0 = half[key]
                        else:
                            ps, k0 = (
                                prj.tile([128, 512], f32, name="ps", tag="prj"),
                                0,
                            )
                        k1 = k0 + KT // 4
                        qk_part(*key, ps, k0, k1)
                        if k1 == KT:
                            done.add(key)
                            half.pop(key, None)
                        else:
                            half[key] = (ps, k1)
                            pending.appendleft(key)
                    else:
                        _, tt, nt, tail = item
                        if (tt, nt) in oprog:
                            o_fin(tt, nt, tail, oprog.pop((tt, nt)))
                        else:
                            oprog[(tt, nt)] = o_begin(tt, nt)
                            pending.appendleft(item)

                done.add(("k", 0, 0))
                done.add(("q", 0, 0))
                qk_first("k", 0)
                qk_first("k", 1)
                for tt in range(4):
                    need("v", tt)
                qk_first("q", 0)
                qk_first("q", 1)
                # more v chains ride the x token-chunk DMA stream on the
                # otherwise idle PE; the rest stream in-loop
                for tt in range(4, TOKT // 2):
                    need("v", tt)
                for ts in range(1, QT):
                    pending.append(("k", 0, ts))
                for ts in range(QT):
                    pending.append(("k", 1, ts))
                pending.append(("q", 1, 0))

                for qt in range(QT):
                    for hp in range(nh // 2):
                        mt = hp
                        if hp == 0 and qt + 1 < QT:
                            pending.append(("q", 0, qt + 1))
                            pending.append(("q", 1, qt + 1))
                        need("k", mt, 0)
                        need("q", mt, qt)
                        o16pair = nrm.tile(
                            [128, 4, 128], bf16, name="o16", tag="o16"
                        )
                        oaccs = [
                            accp.tile([128, 4, VW], f32, name="oacc", tag="acc")
                            for _ in range(2)
                        ]

                        def pv(oa, h, pt, m):
                            last = m == TOKT // 2 - 1
                            for j in range(2):
                                for qi in range(4):
                                    nc.tensor.matmul(
                                        oa[:, qi, :],
                                        lhsT=pt[:, j, qi * 128 : (qi + 1) * 128],
                                        rhs=v1[:, 2 * m + j, VW * h : VW * h + VW],
                                        start=False,
                                        stop=(last and j == 1),
                                        skip_group_check=True,
                                    )

                        pv_wait = None
                        for m in range(TOKT // 2):
                            need("k", mt, m // 2)
                            for hh in range(2):
                                h = 2 * hp + hh
                                bp = 64 * hh
                                n0 = len(done)
                                if pending and len(done) == n0:
                                    emit(pending.popleft())
                                st = stp.tile(
                                    [128, 2, 512], f32, name="st", tag="st"
                                )
                                for j in range(2):
                                    nc.tensor.matmul(
                                        st[:, j, :],
                                        lhsT=kT_sb[
                                            bp : bp + 64,
                                            mt,
                                            (2 * m + j) * 128
                                            : (2 * m + j + 1) * 128,
                                        ],
                                        rhs=qT_sb[
                                            bp : bp + 64,
                                            mt,
                                            qt * 512 : (qt + 1) * 512,
                                        ],
                                        start=True,
                                        stop=True,
                                    )
                                pt = ptp.tile(
                                    [128, 2, 512], bf16, name="pt", tag="pt"
                                )
                                nc.scalar.activation(pt[:], st[:], Exp, scale=scale)
                                if m == 0 and hh == 0:
                                    # zero-open both accumulators here, behind
                                    # the first S/exp, so they don't head-of-
                                    # line block PE on the previous pair's
                                    # normalize
                                    for oa in oaccs:
                                        nc.tensor.matmul(
                                            oa[:],
                                            lhsT=zeros16[:],
                                            rhs=dumm16[:],
                                            start=True,
                                            stop=False,
                                            skip_group_check=True,
                                        )
                                need("v", 2 * m + hh)
                                if pv_wait is not None:
                                    pv(*pv_wait)
                                pv_wait = (oaccs[hh], h, pt, m)
                        def norm(hh):
                            bp = 64 * hh
                            oa = oaccs[hh]
                            rcp = nrm.tile([128, 4, 1], f32, name="rcp", tag="rcp")
                            nc.vector.reciprocal(rcp[:], oa[:, :, 64:65])
                            nc.vector.tensor_tensor(
                                out=o16pair[:, :, bp : bp + 64],
                                in0=oa[:, :, 0:64],
                                in1=rcp[:].to_broadcast((128, 4, 64)),
                                op=mybir.AluOpType.mult,
                            )

                        # head hh0's PVs are all emitted (deferral means the
                        # pending one is hh1's last) -> start its normalize on
                        # DVE while PE finishes hh1's last PV
                        norm(0)
                        pv(*pv_wait)
                        norm(1)
                        for qi in range(4):
                            nc.sync.dma_start_transpose(
                                oT_sb[:, mt, qt, qi, :], o16pair[:, qi, :]
                            )
                        if mt == 1:
                            for tt in range(
                                qt * (TOKT // QT), (qt + 1) * (TOKT // QT)
                            ):
                                for nt in range(DO // 512):
                                    pending.append(("o", tt, nt, qt == QT - 1))
                while pending:
                    emit(pending.popleft())
    nc.finalize()
    return nc


def make_in_maps(x, W_qkv, b_qkv, W_o):
    """Shard full inputs into per-core input maps (core c: batch c//4, group c%4)."""
    x = np.asarray(x, dtype=np.float32)
    W_qkv = np.asarray(W_qkv, dtype=np.float32)
    b_qkv = np.asarray(b_qkv, dtype=np.float32)
    W_o = np.asarray(W_o, dtype=np.float32)
    bf16 = ml_dtypes.bfloat16
    KT = D // 128

    def fold(a):  # [D, C] -> [128, KT, C] bf16
        return np.ascontiguousarray(a.reshape(KT, 128, -1).transpose(1, 0, 2)).astype(
            bf16
        )

    in_maps = []
    for c in range(NCORES):
        b, g = divmod(c, GROUPS)
        cols = slice(CH * g, CH * (g + 1))
        Wv = W_qkv[:, 2 * D : 3 * D][:, cols]
        bv = b_qkv[2 * D : 3 * D][cols]
        Wv_pad = np.zeros((D, NH * VW), dtype=np.float32)
        bv_pad = np.zeros((NH * VW,), dtype=np.float32)
        for h in range(NH):
            Wv_pad[:, VW * h : VW * h + SUB] = Wv[:, SUB * h : SUB * (h + 1)]
            bv_pad[VW * h : VW * h + SUB] = bv[SUB * h : SUB * (h + 1)]
            bv_pad[VW * h + SUB] = 1.0
        aux = np.concatenate([np.ones(128, np.float32), bv_pad])
        bqk = np.stack(
            [
                b_qkv[0 * D : 1 * D][cols].reshape(CH // 128, 128).T,
                b_qkv[1 * D : 2 * D][cols].reshape(CH // 128, 128).T,
            ],
            axis=1,
        )
        xt = x[b].T.reshape(D // 128, 128, N // 256, 256)
        m = {
            "xT": np.ascontiguousarray(xt.transpose(1, 2, 0, 3)).astype(bf16),
            "wq": fold(W_qkv[:, 0 * D : 1 * D][:, cols]),
            "wk": fold(W_qkv[:, 1 * D : 2 * D][:, cols]),
            "wv": fold(Wv_pad),
            "aux": aux[None, :].astype(bf16),
            "wo": np.ascontiguousarray(W_o[cols, :]).astype(bf16),
            "bqk": np.ascontiguousarray(bqk),
        }
        in_maps.append(m)
    return in_maps


_NC = None


def get_nc():
    global _NC
    if _NC is None:
        _NC = build_nc()
    return _NC


def kernel(x, W_qkv, b_qkv, W_o, b_o):
    from concourse import bass_utils

    b_o = np.asarray(b_o, dtype=np.float32)
    in_maps = make_in_maps(x, W_qkv, b_qkv, W_o)
    res = bass_utils.run_bass_kernel_spmd(get_nc(), in_maps, core_ids=list(range(NCORES)))
    out = np.empty((B, N, D), dtype=np.float32)
    for b in range(B):
        acc = res.results[4 * b]["out"].astype(np.float32)
        for g in range(1, GROUPS):
            acc += res.results[4 * b + g]["out"].astype(np.float32)
        out[b] = acc + b_o
    return out
